# revision 1
# baseline (speedup 1.0000x reference)
"""Trainium2 Bass kernel for CapsuleLayer dynamic routing (B=128, I=1152, J=128, K=32, D=32).

Strategy
--------
Data-parallel over batch: 16 samples per core x 8 cores. The routing math is
algebraically factorized so u_hat [B,I,K,D] (604 MB) is never materialized:

    y[s,k,j]  = sum_i c[s,i,k] x[s,i,j]          (per-sample PE matmul, i contracted)
    s[s,k,d]  = sum_j y[s,k,j] W[j,k,d]          (per-k PE matmul, j contracted)
    v         = squash(s)                         (DVE/ACT elementwise)
    t[s,j,k]  = sum_d W[j,k,d] v[s,k,d]          (per-k PE matmul, d contracted)
    b[s,i,k] += sum_j x[s,i,j] t[s,j,k]          (per-sample PE matmul, j contracted)

x is staged in two on-chip layouts (i-major for y, j-major for the b update),
both prepared host-side in bf16; the high-traffic y / b-update matmuls run in
bf16, the small s / t matmuls in fp32r (single-pass), all with fp32 PSUM
accumulate; routing state (b, softmax, squash) stays fp32. Iteration 0's
softmax of zeros is uniform, so its y reduces to sum_i x / K, computed
host-side in fp32 (y0). Softmax runs in 4-sample groups so its serial chain
pipelines against the PE matmul stream.
"""
import numpy as np
import ml_dtypes
from contextlib import ExitStack

import concourse.bass as bass
import concourse.bacc as bacc_mod
import concourse.mybir as mybir
import concourse.tile as tile
from concourse.bass_utils import run_bass_kernel_spmd
from concourse.masks import make_identity

B, I, J, K, D = 128, 1152, 128, 32, 32
NCORES = 8
S = B // NCORES          # 16 samples per core
CH = I // 128            # 9 chunks of the input-capsule axis
NUM_ROUTING = 3
EPS = 1e-7
F32 = mybir.dt.float32
F32R = mybir.dt.float32r
BF16 = mybir.dt.bfloat16

_PROGRAM = None

SG = 4  # softmax sample-group size


def _softmax_group(nc, pools, btile, g):
    """softmax over k of b[:, g*SG:(g+1)*SG] -> bf16 [128, SG, CH, K] tile."""
    cpool, zpool = pools
    bsl = btile[:, g * SG:(g + 1) * SG]
    cs = cpool.tile([128, SG, CH, K], BF16, tag="cs")
    nc.scalar.activation(out=cs, in_=bsl, func=mybir.ActivationFunctionType.Exp)
    z = zpool.tile([128, SG, CH], F32, tag="z")
    nc.vector.tensor_reduce(out=z, in_=cs, axis=mybir.AxisListType.X,
                            op=mybir.AluOpType.add)
    r = zpool.tile([128, SG, CH], F32, tag="r")
    nc.vector.reciprocal(out=r, in_=z)
    nc.vector.tensor_tensor(out=cs, in0=cs,
                            in1=r.unsqueeze(-1).broadcast_to([128, SG, CH, K]),
                            op=mybir.AluOpType.mult)
    return cs


def _squash(nc, sqpool, s_ps, vsb, eps_t, k0, k1):
    """vsb[:, k0:k1] = squash(s_ps[:, k0:k1]) along d (k-slice for pipelining)."""
    kw = k1 - k0
    sp = s_ps[:, k0:k1, :]
    sq = sqpool.tile([S, kw, D], F32, tag=f"sq{k0}")
    # Square on ACT: single PSUM read (two PSUM reads on DVE are illegal)
    nc.scalar.activation(out=sq, in_=sp, func=mybir.ActivationFunctionType.Square)
    ss = sqpool.tile([S, kw], F32, tag=f"ss{k0}")
    nc.vector.tensor_reduce(out=ss, in_=sq, axis=mybir.AxisListType.X,
                            op=mybir.AluOpType.add)
    rt = sqpool.tile([S, kw], F32, tag=f"rt{k0}")      # sqrt(ss + eps)
    nc.scalar.activation(out=rt, in_=ss, func=mybir.ActivationFunctionType.Sqrt,
                         bias=eps_t)
    den = sqpool.tile([S, kw], F32, tag=f"den{k0}")    # (1 + ss) * sqrt(ss + eps)
    nc.vector.scalar_tensor_tensor(out=den, in0=ss, scalar=1.0, in1=rt,
                                   op0=mybir.AluOpType.add, op1=mybir.AluOpType.mult)
    rden = sqpool.tile([S, kw], F32, tag=f"rden{k0}")
    nc.vector.reciprocal(out=rden, in_=den)
    sc = sqpool.tile([S, kw], F32, tag=f"sc{k0}")      # ss / ((1+ss)*sqrt(ss+eps))
    nc.vector.tensor_mul(sc, ss, rden)
    nc.vector.tensor_tensor(out=vsb[:, k0:k1, :], in0=sp,
                            in1=sc.unsqueeze(-1).broadcast_to([S, kw, D]),
                            op=mybir.AluOpType.mult)


def _build_program():
    nc = bacc_mod.Bacc("TRN2", target_bir_lowering=False, debug=False,
                       num_devices=NCORES)
    xa_d = nc.dram_tensor("xa", [128, S, CH, 128], BF16, kind="ExternalInput")
    xb_d = nc.dram_tensor("xb", [128, S, CH * 128], BF16, kind="ExternalInput")
    wr_d = nc.dram_tensor("wr", [128, K, D], F32R, kind="ExternalInput")
    wt_d = nc.dram_tensor("wt", [32, K, 128], F32R, kind="ExternalInput")
    y0_d = nc.dram_tensor("y0", [128, S], F32R, kind="ExternalInput")
    v_d = nc.dram_tensor("vout", [S, K, D], F32, kind="ExternalOutput")

    with tile.TileContext(nc) as tc, ExitStack() as ctx:
        const = ctx.enter_context(tc.tile_pool(name="const", bufs=1))
        xap = ctx.enter_context(tc.tile_pool(name="xa", bufs=1))
        xbp = ctx.enter_context(tc.tile_pool(name="xb", bufs=1))
        bp = ctx.enter_context(tc.tile_pool(name="b", bufs=1))
        cpool = ctx.enter_context(tc.tile_pool(name="c", bufs=3))
        zpool = ctx.enter_context(tc.tile_pool(name="z", bufs=3))
        y2p = ctx.enter_context(tc.tile_pool(name="y2", bufs=2))
        t2p = ctx.enter_context(tc.tile_pool(name="t2", bufs=2))
        vt2p = ctx.enter_context(tc.tile_pool(name="vt2", bufs=2))
        vp = ctx.enter_context(tc.tile_pool(name="v", bufs=1))
        sqp = ctx.enter_context(tc.tile_pool(name="sqp", bufs=1))
        ps_y = ctx.enter_context(tc.tile_pool(name="ps_y", bufs=1, space="PSUM"))
        ps_s = ctx.enter_context(tc.tile_pool(name="ps_s", bufs=1, space="PSUM"))
        ps_vt = ctx.enter_context(tc.tile_pool(name="ps_vt", bufs=1, space="PSUM"))
        ps_t = ctx.enter_context(tc.tile_pool(name="ps_t", bufs=1, space="PSUM"))
        ps_b = ctx.enter_context(tc.tile_pool(name="ps_b", bufs=3, space="PSUM"))

        # constants (one DMA queue)
        wr = const.tile([128, K, D], F32R)
        nc.sync.dma_start(out=wr, in_=wr_d[:])
        wt = const.tile([32, K, 128], F32R)
        nc.sync.dma_start(out=wt, in_=wt_d[:])
        y0t = const.tile([128, S], F32R)
        nc.sync.dma_start(out=y0t, in_=y0_d[:])
        ident = const.tile([S, S], F32)
        make_identity(nc, ident)
        eps_t = const.tile([S, 1], F32)
        nc.vector.memset(eps_t, EPS)

        # x loads: xb gates iteration 0, xa gates iteration 1.
        # Spread across engine queues for DMA parallelism.
        # Interleave xb/xa per sample across the two HWDGE queues: sample s's
        # pair lands together, so iteration 1's y-matmuls for early sample
        # groups can start while later samples are still in flight.
        dma_engines = [nc.sync, nc.scalar]
        xb_s = []
        xa_s = []
        for s in range(S):
            tb = xbp.tile([128, CH * 128], BF16, tag=f"xb{s}")
            dma_engines[s % 2].dma_start(out=tb, in_=xb_d[:, s])
            xb_s.append(tb)
            ta = xap.tile([128, CH, 128], BF16, tag=f"xa{s}")
            dma_engines[(s + 1) % 2].dma_start(out=ta, in_=xa_d[:, s])
            xa_s.append(ta)

        btile = bp.tile([128, S, CH, K], F32)
        wrf = wr.rearrange("p k d -> p (k d)")

        for it in range(NUM_ROUTING):
            # ---- s[s,k,d] in PSUM [S, K, D] f32
            s_ps = ps_s.tile([S, K, D], F32, tag="s")
            s_flat = s_ps.rearrange("s k d -> s (k d)")
            if it == 0:
                nc.tensor.matmul(s_flat[:, :512], y0t,
                                 wrf[:, :512], start=True, stop=True)
                nc.tensor.matmul(s_flat[:, 512:], y0t,
                                 wrf[:, 512:], start=True, stop=True)
            else:
                # y^T for all samples into one PSUM bank, then one copy out
                y_ps = ps_y.tile([128, S, K], F32, tag="y")
                for g in range(S // SG):
                    cs = _softmax_group(nc, (cpool, zpool), btile, g)
                    for si in range(SG):
                        s = g * SG + si
                        for ic in range(CH):
                            nc.tensor.matmul(y_ps[:, s, :], xa_s[s][:, ic, :],
                                             cs[:, si, ic, :],
                                             start=(ic == 0), stop=(ic == CH - 1))
                Y2 = y2p.tile([128, S, K], F32R, tag="Y2")
                nc.vector.tensor_copy(out=Y2, in_=y_ps)
                for k in range(K):
                    nc.tensor.matmul(s_ps[:, k, :], Y2[:, :, k],
                                     wr[:, k, :],
                                     start=True, stop=True)

            # ---- v = squash(s)
            vsb = vp.tile([S, K, D], F32, tag="v")
            _squash(nc, sqp, s_ps, vsb, eps_t, 0, K)

            if it == NUM_ROUTING - 1:
                nc.sync.dma_start(out=v_d[:], in_=vsb)
                break

            # ---- t[s,j,k] then b update
            vt_ps = ps_vt.tile([32, K, S], F32, tag="vt")
            for k in range(K):
                nc.tensor.transpose(out=vt_ps[:, k, :], in_=vsb[:, k, :],
                                    identity=ident)
            vt2 = vt2p.tile([32, K, S], F32R, tag="vt2")
            nc.vector.tensor_copy(out=vt2, in_=vt_ps)
            t_ps = ps_t.tile([128, K, S], F32, tag="t")
            for k in range(K):
                nc.tensor.matmul(t_ps[:, k, :], wt[:, k, :],
                                 vt2[:, k, :], start=True, stop=True)
            T2 = t2p.tile([128, K, S], BF16, tag="T2")
            nc.vector.tensor_copy(out=T2, in_=t_ps)
            for s in range(S):
                bu = ps_b.tile([128, CH, K], F32, tag="bu")
                for ic in range(CH):
                    nc.tensor.matmul(bu[:, ic, :],
                                     xb_s[s][:, ic * 128:(ic + 1) * 128],
                                     T2[:, :, s], start=True, stop=True)
                if it == 0:
                    nc.vector.tensor_copy(out=btile[:, s], in_=bu)
                else:
                    nc.vector.tensor_add(btile[:, s], btile[:, s], bu)

    nc.compile()
    return nc


def _get_program():
    global _PROGRAM
    if _PROGRAM is None:
        _PROGRAM = _build_program()
    return _PROGRAM


def _prep_core_inputs(x_core, wr, wt):
    """x_core: [S, I, J] fp32 -> per-core input map."""
    bf = ml_dtypes.bfloat16
    xa = np.ascontiguousarray(
        x_core.reshape(S, CH, 128, J).transpose(2, 0, 1, 3).astype(bf))  # [128,S,CH,J]
    xb = np.ascontiguousarray(x_core.transpose(2, 0, 1).astype(bf))      # [J,S,I]
    y0 = np.ascontiguousarray((x_core.sum(axis=1) / K).T)                # [J,S] f32
    return {"xa": xa, "xb": xb.reshape(J, S, CH * 128), "wr": wr, "wt": wt, "y0": y0}


def kernel(inputs, W):
    x = np.ascontiguousarray(np.asarray(inputs, dtype=np.float32))
    Wf = np.ascontiguousarray(np.asarray(W, dtype=np.float32))           # [J, K, D]
    wt = np.ascontiguousarray(Wf.transpose(2, 1, 0))                     # [D, K, J]
    nc = _get_program()
    in_maps = [_prep_core_inputs(x[c * S:(c + 1) * S], Wf, wt) for c in range(NCORES)]
    res = run_bass_kernel_spmd(nc, in_maps, list(range(NCORES)))
    return np.concatenate([r["vout"] for r in res.results], axis=0)



# revision 6
# speedup vs baseline: 1.1225x; 1.1225x over previous
"""Trainium2 Bass kernel for CapsuleLayer dynamic routing (B=128, I=1152, J=128, K=32, D=32).

Strategy
--------
Data-parallel over batch: 16 samples per core x 8 cores. The routing math is
algebraically factorized so u_hat [B,I,K,D] (604 MB) is never materialized:

    y[s,k,j]  = sum_i c[s,i,k] x[s,i,j]          (per-sample PE matmul, i contracted)
    s[s,k,d]  = sum_j y[s,k,j] W[j,k,d]          (per-k PE matmul, j contracted)
    v         = squash(s)                         (DVE/ACT elementwise)
    t[s,j,k]  = sum_d W[j,k,d] v[s,k,d]          (per-k PE matmul, d contracted)
    b[s,i,k] += sum_j x[s,i,j] t[s,j,k]          (per-sample PE matmul, j contracted)

x is staged in two on-chip layouts (i-major for y, j-major for the b update),
both prepared host-side in bf16. Perf-critical structure (from trace analysis):
  - all big matmuls keep CONTIGUOUS moving operands (strided moving costs 60ns
    vs 27ns per instr): cs is k-inner, T2 is [j, S, K] so T2[:, s, :] is flat.
  - t-matmuls run in bf16 (f32r stationary ldweights cost 328ns each).
  - PSUM->SBUF b copies alternate DVE/ACT so the PE never stalls on a copy.
  - x DMAs ride 4 hardware queues (sync/gpsimd/vector/scalar), ordered so
    group g's xb then xa land before group g+1 (group-wise pipelining of
    iteration 0's b-update and iteration 1's softmax+y).
  - iteration 1's b-update is emitted interleaved with iteration 2's softmax
    groups, y(it2, g) after b-matmuls(g+1), so PE/ACT/DVE/Pool all pipeline.
  - softmax normalize mult alternates DVE/Pool (GpSimd) per group.
Iteration 0's softmax of zeros is uniform, so its y reduces to sum_i x / K,
computed host-side in fp32 (y0).
"""
import numpy as np
import ml_dtypes
from contextlib import ExitStack

import concourse.bass as bass
import concourse.bacc as bacc_mod
import concourse.mybir as mybir
import concourse.tile as tile
from concourse.bass_utils import run_bass_kernel_spmd
from concourse.masks import make_identity

B, I, J, K, D = 128, 1152, 128, 32, 32
NCORES = 8
S = B // NCORES          # 16 samples per core
CH = I // 128            # 9 chunks of the input-capsule axis
NUM_ROUTING = 3
EPS = 1e-7
F32 = mybir.dt.float32
F32R = mybir.dt.float32r
BF16 = mybir.dt.bfloat16

_PROGRAM = None

SG = 4  # softmax sample-group size
NG = S // SG


def _softmax_group(nc, pools, btile, g):
    """softmax over k of b[:, g*SG:(g+1)*SG] -> bf16 [128, SG, CH, K] tile."""
    cpool, zpool = pools
    bsl = btile[:, g * SG:(g + 1) * SG]
    cs = cpool.tile([128, SG, CH, K], BF16, tag=f"cs{g % 2}")
    nc.scalar.activation(out=cs, in_=bsl, func=mybir.ActivationFunctionType.Exp)
    z = zpool.tile([128, SG, CH], F32, tag=f"z{g % 2}")
    nc.vector.tensor_reduce(out=z, in_=cs, axis=mybir.AxisListType.X,
                            op=mybir.AluOpType.add)
    r = zpool.tile([128, SG, CH], F32, tag=f"r{g % 2}")
    nc.vector.reciprocal(out=r, in_=z)
    eng = nc.vector if g % 2 == 0 else nc.gpsimd
    eng.tensor_tensor(out=cs, in0=cs,
                      in1=r.unsqueeze(-1).broadcast_to([128, SG, CH, K]),
                      op=mybir.AluOpType.mult)
    return cs


def _squash(nc, sqpool, s_ps, vsb, eps_t, k0, k1):
    """vsb[:, k0:k1] = squash(s_ps[:, k0:k1]) along d (k-slice for pipelining)."""
    kw = k1 - k0
    sp = s_ps[:, k0:k1, :]
    sq = sqpool.tile([S, kw, D], F32, tag=f"sq{k0}")
    # Square on ACT: single PSUM read (two PSUM reads on DVE are illegal)
    nc.scalar.activation(out=sq, in_=sp, func=mybir.ActivationFunctionType.Square)
    ss = sqpool.tile([S, kw], F32, tag=f"ss{k0}")
    nc.vector.tensor_reduce(out=ss, in_=sq, axis=mybir.AxisListType.X,
                            op=mybir.AluOpType.add)
    rt = sqpool.tile([S, kw], F32, tag=f"rt{k0}")      # sqrt(ss + eps)
    nc.scalar.activation(out=rt, in_=ss, func=mybir.ActivationFunctionType.Sqrt,
                         bias=eps_t)
    den = sqpool.tile([S, kw], F32, tag=f"den{k0}")    # (1 + ss) * sqrt(ss + eps)
    nc.vector.scalar_tensor_tensor(out=den, in0=ss, scalar=1.0, in1=rt,
                                   op0=mybir.AluOpType.add, op1=mybir.AluOpType.mult)
    rden = sqpool.tile([S, kw], F32, tag=f"rden{k0}")
    nc.vector.reciprocal(out=rden, in_=den)
    sc = sqpool.tile([S, kw], F32, tag=f"sc{k0}")      # ss / ((1+ss)*sqrt(ss+eps))
    nc.vector.tensor_mul(sc, ss, rden)
    nc.vector.tensor_tensor(out=vsb[:, k0:k1, :], in0=sp,
                            in1=sc.unsqueeze(-1).broadcast_to([S, kw, D]),
                            op=mybir.AluOpType.mult)


def _build_program():
    nc = bacc_mod.Bacc("TRN2", target_bir_lowering=False, debug=False,
                       num_devices=NCORES)
    xa_d = nc.dram_tensor("xa", [128, S, CH, 128], BF16, kind="ExternalInput")
    xb_d = nc.dram_tensor("xb", [128, S, CH * 128], BF16, kind="ExternalInput")
    wr_d = nc.dram_tensor("wr", [128, K, D], F32R, kind="ExternalInput")
    wt_d = nc.dram_tensor("wt", [32, K, 128], BF16, kind="ExternalInput")
    y0_d = nc.dram_tensor("y0", [128, S], F32R, kind="ExternalInput")
    v_d = nc.dram_tensor("vout", [S, K, D], F32, kind="ExternalOutput")

    with tile.TileContext(nc) as tc, ExitStack() as ctx:
        const = ctx.enter_context(tc.tile_pool(name="const", bufs=1))
        xap = ctx.enter_context(tc.tile_pool(name="xa", bufs=1))
        xbp = ctx.enter_context(tc.tile_pool(name="xb", bufs=1))
        bp = ctx.enter_context(tc.tile_pool(name="b", bufs=1))
        cpool = ctx.enter_context(tc.tile_pool(name="c", bufs=2))
        zpool = ctx.enter_context(tc.tile_pool(name="z", bufs=2))
        y2p = ctx.enter_context(tc.tile_pool(name="y2", bufs=2))
        t2p = ctx.enter_context(tc.tile_pool(name="t2", bufs=2))
        vt2p = ctx.enter_context(tc.tile_pool(name="vt2", bufs=2))
        vp = ctx.enter_context(tc.tile_pool(name="v", bufs=1))
        sqp = ctx.enter_context(tc.tile_pool(name="sqp", bufs=1))
        # y and s PSUM tiles have disjoint lifetimes (y -> Y2 copy -> s) so
        # they share one 2-bank pool slot.
        ps_ys = ctx.enter_context(tc.tile_pool(name="ps_ys", bufs=1, space="PSUM"))
        ps_y = ps_s = ps_ys
        ps_vt = ctx.enter_context(tc.tile_pool(name="ps_vt", bufs=1, space="PSUM"))
        ps_t = ctx.enter_context(tc.tile_pool(name="ps_t", bufs=1, space="PSUM"))
        ps_b = ctx.enter_context(tc.tile_pool(name="ps_b", bufs=3, space="PSUM"))

        # constants: small, needed first for iteration 0's s/t chain.
        y0t = const.tile([128, S], F32R)
        nc.sync.dma_start(out=y0t, in_=y0_d[:])
        wr = const.tile([128, K, D], F32R)
        nc.gpsimd.dma_start(out=wr, in_=wr_d[:])
        wt = const.tile([32, K, 128], BF16)
        nc.sync.dma_start(out=wt, in_=wt_d[:])
        ident = const.tile([S, S], F32)
        make_identity(nc, ident)
        eps_t = const.tile([S, 1], F32)
        nc.vector.memset(eps_t, EPS)

        # x loads on 4 hardware queues, ordered group-wise: group g's xb
        # tiles land, then its xa tiles, then group g+1 — so iteration 0's
        # b-update and iteration 1's softmax+y pipeline behind the DMA.
        xb_s = [None] * S
        xa_s = [None] * S
        qi = 0
        for g in range(NG):
            # early groups ride sync+gpsimd (scalar's engine has it0 compute);
            # late groups use all three queues.
            dq = [nc.sync, nc.gpsimd] if g < 2 else [nc.sync, nc.gpsimd, nc.scalar]
            for si in range(SG):
                s = g * SG + si
                tb = xbp.tile([128, CH * 128], BF16, tag=f"xb{s}")
                dq[qi % len(dq)].dma_start(out=tb, in_=xb_d[:, s])
                qi += 1
                xb_s[s] = tb
            for si in range(SG):
                s = g * SG + si
                ta = xap.tile([128, CH, 128], BF16, tag=f"xa{s}")
                dq[qi % len(dq)].dma_start(out=ta, in_=xa_d[:, s])
                qi += 1
                xa_s[s] = ta

        btile = bp.tile([128, S, CH, K], F32)
        wrf = wr.rearrange("p k d -> p (k d)")

        def s_matmuls(Y2, tag):
            s_ps = ps_s.tile([S, K, D], F32, tag="s")
            for k in range(K):
                nc.tensor.matmul(s_ps[:, k, :], Y2[:, :, k], wr[:, k, :],
                                 start=True, stop=True)
            return s_ps

        def squash_to_v(s_ps):
            vsb = vp.tile([S, K, D], F32, tag="v")
            _squash(nc, sqp, s_ps, vsb, eps_t, 0, K // 2)
            _squash(nc, sqp, s_ps, vsb, eps_t, K // 2, K)
            return vsb

        def v_to_T2(vsb):
            # vT[d, k, s] via PE transposes, t[j, k, s] matmuls in bf16,
            # then cast with free-dim transpose to T2 [j, s, k] so the
            # b-update moving operand T2[:, s, :] is contiguous.
            vt_ps = ps_vt.tile([32, K, S], F32, tag="vt")
            for k in range(K):
                nc.tensor.transpose(out=vt_ps[:, k, :], in_=vsb[:, k, :],
                                    identity=ident)
            vt2 = vt2p.tile([32, K, S], BF16, tag="vt2")
            nc.scalar.copy(out=vt2, in_=vt_ps)
            t_ps = ps_t.tile([128, K, S], F32, tag="t")
            for k in range(K):
                nc.tensor.matmul(t_ps[:, k, :], wt[:, k, :],
                                 vt2[:, k, :], start=True, stop=True)
            T2 = t2p.tile([128, S, K], BF16, tag="T2")
            nc.vector.tensor_copy(out=T2.rearrange("p s k -> p k s"), in_=t_ps)
            return T2

        def b_update(s, T2, it):
            bu = ps_b.tile([128, CH, K], F32, tag="bu")
            for ic in range(CH):
                nc.tensor.matmul(bu[:, ic, :],
                                 xb_s[s][:, ic * 128:(ic + 1) * 128],
                                 T2[:, s, :], start=True, stop=True)
            buf = bu.rearrange("p c k -> p (c k)")
            dst = btile[:, s].rearrange("p c k -> p (c k)")
            if it == 0:
                # copies alternate DVE/ACT so the PE never waits on one engine
                if s % 2 == 0:
                    nc.vector.tensor_copy(out=dst, in_=buf)
                else:
                    nc.scalar.copy(out=dst, in_=buf)
            else:
                nc.vector.tensor_tensor(out=dst, in0=dst, in1=buf,
                                        op=mybir.AluOpType.add)

        def y_matmuls(y_ps, cs, g):
            for si in range(SG):
                s = g * SG + si
                for ic in range(CH):
                    nc.tensor.matmul(y_ps[:, s, :], xa_s[s][:, ic, :],
                                     cs[:, si, ic, :],
                                     start=(ic == 0), stop=(ic == CH - 1))

        # ---------------- iteration 0 ----------------
        s_ps = ps_s.tile([S, K, D], F32, tag="s")
        s_flat = s_ps.rearrange("s k d -> s (k d)")
        nc.tensor.matmul(s_flat[:, :512], y0t, wrf[:, :512], start=True, stop=True)
        nc.tensor.matmul(s_flat[:, 512:], y0t, wrf[:, 512:], start=True, stop=True)
        vsb = squash_to_v(s_ps)
        T2 = v_to_T2(vsb)
        for s in range(S):
            b_update(s, T2, it=0)

        # ---------------- iteration 1: softmax + y, then s/squash/t ----------
        y_ps = ps_y.tile([128, S, K], F32, tag="y")
        for g in range(NG):
            cs = _softmax_group(nc, (cpool, zpool), btile, g)
            y_matmuls(y_ps, cs, g)
        Y2 = y2p.tile([128, S, K], F32R, tag="Y2")
        nc.vector.tensor_copy(out=Y2, in_=y_ps)
        s_ps = s_matmuls(Y2, "s1")
        vsb = squash_to_v(s_ps)
        T2 = v_to_T2(vsb)

        # ---------------- iteration 1 b-update interleaved with it2 softmax --
        # PE order: b(g0), b(g1), y2(g0), b(g2), y2(g1), b(g3), y2(g2), y2(g3)
        # so the PE never stalls on a softmax chain.
        y_ps2 = ps_y.tile([128, S, K], F32, tag="y")
        cs_q = []
        for g in range(NG):
            for si in range(SG):
                b_update(g * SG + si, T2, it=1)
            cs_q.append(_softmax_group(nc, (cpool, zpool), btile, g))
            if g >= 1:
                y_matmuls(y_ps2, cs_q[g - 1], g - 1)
        y_matmuls(y_ps2, cs_q[NG - 1], NG - 1)

        # ---------------- iteration 2 tail: s, squash, output ----------------
        Y2b = y2p.tile([128, S, K], F32R, tag="Y2b")
        nc.vector.tensor_copy(out=Y2b, in_=y_ps2)
        s_ps = s_matmuls(Y2b, "s2")
        vsb = squash_to_v(s_ps)
        nc.sync.dma_start(out=v_d[:], in_=vsb)

    nc.compile()
    return nc


def _get_program():
    global _PROGRAM
    if _PROGRAM is None:
        _PROGRAM = _build_program()
    return _PROGRAM


def _prep_core_inputs(x_core, wr, wt):
    """x_core: [S, I, J] fp32 -> per-core input map."""
    bf = ml_dtypes.bfloat16
    xa = np.ascontiguousarray(
        x_core.reshape(S, CH, 128, J).transpose(2, 0, 1, 3).astype(bf))  # [128,S,CH,J]
    xb = np.ascontiguousarray(x_core.transpose(2, 0, 1).astype(bf))      # [J,S,I]
    y0 = np.ascontiguousarray((x_core.sum(axis=1) / K).T)                # [J,S] f32
    return {"xa": xa, "xb": xb.reshape(J, S, CH * 128), "wr": wr,
            "wt": np.ascontiguousarray(wt.astype(bf)), "y0": y0}


def kernel(inputs, W):
    x = np.ascontiguousarray(np.asarray(inputs, dtype=np.float32))
    Wf = np.ascontiguousarray(np.asarray(W, dtype=np.float32))           # [J, K, D]
    wt = np.ascontiguousarray(Wf.transpose(2, 1, 0))                     # [D, K, J]
    nc = _get_program()
    in_maps = [_prep_core_inputs(x[c * S:(c + 1) * S], Wf, wt) for c in range(NCORES)]
    res = run_bass_kernel_spmd(nc, in_maps, list(range(NCORES)))
    return np.concatenate([r["vout"] for r in res.results], axis=0)
